# revision 1
# baseline (speedup 1.0000x reference)
"""v5: strided-rhs implicit-GEMM conv, all-sync input DMAs, t-outer loop.

vs v2: x DMAs move scalar->sync (HWDGE, fast) with xin bufs=1 so image
1-3 prefetches are gated by slot reuse and cannot inflate the first
matmul's DMA wait; loop order is t-outer/h-inner so the xa slot releases
~23% into each image, giving the next image's prefetch a wide margin.
"""

import sys

if "/opt/trn_rl_repo" not in sys.path:
    sys.path.insert(0, "/opt/trn_rl_repo")

import numpy as np

N, C_IN, H, W = 32, 128, 56, 56
C_OUT, KH, KW = 256, 3, 3
N_CORES = 8
IMGS = N // N_CORES
HP, WP = H + 2, W + 2
RPT = 8
NT = H // RPT          # 7
TF = RPT * W           # 448
NH = C_OUT // 128      # 2

XA_R0, XA_R1 = 0, 26   # rows for tiles t=0..2
XB_R0, XB_R1 = 24, 58  # rows for tiles t=3..6
T_SPLIT = 3
OUT_SPLIT = 4          # chunk A = tiles 0..3
N_WARMUP_MM = 18

_CACHE = {}


def _build_program():
    import concourse.mybir as mybir
    import concourse.tile as tile
    from concourse import bacc

    F32 = mybir.dt.float32
    F32R = mybir.dt.float32r

    nc = bacc.Bacc("TRN2", target_bir_lowering=False, debug=False,
                   enable_asserts=False)

    xp = nc.dram_tensor("xp", [IMGS, C_IN, HP, WP], F32R,
                        kind="ExternalInput").ap()
    w = nc.dram_tensor("w", [C_IN, KH * KW, C_OUT], F32R,
                       kind="ExternalInput").ap()
    b = nc.dram_tensor("b", [128, NH], F32, kind="ExternalInput").ap()
    out = nc.dram_tensor("out", [IMGS, C_OUT, H, W], F32,
                         kind="ExternalOutput").ap()
    out_v = out.rearrange("n c a b -> n c (a b)")

    with tile.TileContext(nc) as tc:
        with (
            tc.tile_pool(name="consts", bufs=1) as consts,
            tc.tile_pool(name="xin", bufs=1) as xin,
            tc.tile_pool(name="outp", bufs=2) as outp,
            tc.tile_pool(name="psum", bufs=7, space="PSUM") as psum,
        ):
            scratch = consts.tile([128, TF], F32, tag="scratch")
            nc.gpsimd.memset(scratch[:], 0.0)
            scr_r = scratch[:, :].bitcast(F32R)
            warm_ps = psum.tile([128, TF], F32, tag="warm", bufs=1)
            for _ in range(N_WARMUP_MM):
                nc.tensor.matmul(warm_ps[:, :], lhsT=scr_r[:, :128],
                                 rhs=scr_r[:, :], start=True, stop=True)

            xts = {}
            for img in range(IMGS):
                xa = xin.tile([C_IN, XA_R1 - XA_R0, WP], F32R, tag="xa")
                nc.sync.dma_start(out=xa[:], in_=xp[img, :, XA_R0:XA_R1])
                if img == 0:
                    # w directly after xa0: the first matmul's cumulative
                    # DMA-sem wait then covers exactly [xa0, w]
                    w_sb = consts.tile([C_IN, KH * KW, C_OUT], F32R, tag="w")
                    nc.sync.dma_start(out=w_sb[:], in_=w)
                xb = xin.tile([C_IN, XB_R1 - XB_R0, WP], F32R, tag="xb")
                nc.sync.dma_start(out=xb[:], in_=xp[img, :, XB_R0:XB_R1])
                xts[img] = (xa, xb)
                if img == 0:
                    b_sb = consts.tile([128, NH], F32, tag="b")
                    nc.sync.dma_start(out=b_sb[:], in_=b)

            for img in range(IMGS):
                xa, xb = xts[img]
                ots = [outp.tile([128, H * W], F32, tag=f"ot{h}",
                                 name=f"ot{img}_{h}")
                       for h in range(NH)]
                for t in range(NT):
                    if t < T_SPLIT:
                        src, r_off = xa, XA_R0
                    else:
                        src, r_off = xb, XB_R0
                    r0 = RPT * t - r_off
                    for h in range(NH):
                        pt = psum.tile([128, TF], F32, tag="pt")
                        for k in range(KH * KW):
                            kh, kw = divmod(k, KW)
                            nc.tensor.matmul(
                                pt[:, :],
                                lhsT=w_sb[:, k, h * 128:(h + 1) * 128],
                                rhs=src[:, r0 + kh:r0 + kh + RPT, kw:kw + W],
                                start=(k == 0),
                                stop=(k == KH * KW - 1),
                            )
                        nc.vector.tensor_scalar_add(
                            out=ots[h][:, t * TF:(t + 1) * TF],
                            in0=pt[:, :],
                            scalar1=b_sb[:, h:h + 1],
                        )
                        nc.sync.dma_start(
                            out=out_v[img, h * 128:(h + 1) * 128,
                                      t * TF:(t + 1) * TF],
                            in_=ots[h][:, t * TF:(t + 1) * TF])
    nc.compile()
    return nc


def get_program():
    if "nc" not in _CACHE:
        _CACHE["nc"] = _build_program()
    return _CACHE["nc"]


def make_in_maps(x, weight, bias):
    x = np.asarray(x, dtype=np.float32)
    weight = np.asarray(weight, dtype=np.float32)
    bias = np.asarray(bias, dtype=np.float32)

    xpad = np.zeros((N, C_IN, HP, WP), dtype=np.float32)
    xpad[:, :, 1:1 + H, 1:1 + W] = x
    w_t = np.ascontiguousarray(
        weight.transpose(1, 2, 3, 0).reshape(C_IN, KH * KW, C_OUT))
    b2 = np.ascontiguousarray(bias.reshape(NH, 128).T)

    return [
        {
            "xp": np.ascontiguousarray(xpad[i * IMGS:(i + 1) * IMGS]),
            "w": w_t,
            "b": b2,
        }
        for i in range(N_CORES)
    ]


def kernel(x, weight, bias):
    from concourse.bass_utils import run_bass_kernel_spmd

    nc = get_program()
    in_maps = make_in_maps(x, weight, bias)
    res = run_bass_kernel_spmd(nc, in_maps, core_ids=list(range(N_CORES)))
    return np.concatenate([res.results[i]["out"] for i in range(N_CORES)],
                          axis=0)



# revision 2
# speedup vs baseline: 1.0971x; 1.0971x over previous
"""v6: bf16 implicit-GEMM conv (FWL weight loads), strided-rhs, t-outer loop.

vs v5 (f32r): bf16 lhsT/rhs lets the PE use fast-weight-load, dropping
LDWEIGHTS from ~191ns (fp32, 2 cyc/col, no FWL — it was the critical
path at 211ns/MM) to ~53ns hidden behind the 189ns matmul stream.
Input DMA bytes halve. PSUM accumulation stays fp32; rel err ~2.3e-3.
"""

import sys

if "/opt/trn_rl_repo" not in sys.path:
    sys.path.insert(0, "/opt/trn_rl_repo")

import numpy as np

N, C_IN, H, W = 32, 128, 56, 56
C_OUT, KH, KW = 256, 3, 3
N_CORES = 8
IMGS = N // N_CORES
HP, WP = H + 2, W + 2
RPT = 8
NT = H // RPT          # 7
TF = RPT * W           # 448
NH = C_OUT // 128      # 2

XA_R0, XA_R1 = 0, 26   # rows for tiles t=0..2
XB_R0, XB_R1 = 24, 58  # rows for tiles t=3..6
T_SPLIT = 3
N_WARMUP_MM = 12

_CACHE = {}


def _build_program():
    import concourse.mybir as mybir
    import concourse.tile as tile
    from concourse import bacc

    F32 = mybir.dt.float32
    BF16 = mybir.dt.bfloat16

    nc = bacc.Bacc("TRN2", target_bir_lowering=False, debug=False,
                   enable_asserts=False)

    xp = nc.dram_tensor("xp", [IMGS, C_IN, HP, WP], BF16,
                        kind="ExternalInput").ap()
    w = nc.dram_tensor("w", [C_IN, KH * KW, C_OUT], BF16,
                       kind="ExternalInput").ap()
    b = nc.dram_tensor("b", [128, NH], F32, kind="ExternalInput").ap()
    out = nc.dram_tensor("out", [IMGS, C_OUT, H, W], F32,
                         kind="ExternalOutput").ap()
    out_v = out.rearrange("n c a b -> n c (a b)")

    with tile.TileContext(nc) as tc:
        with (
            tc.tile_pool(name="consts", bufs=1) as consts,
            tc.tile_pool(name="xin", bufs=1) as xin,
            tc.tile_pool(name="outp", bufs=2) as outp,
            tc.tile_pool(name="psum", bufs=7, space="PSUM") as psum,
        ):
            scratch = consts.tile([128, TF], BF16, tag="scratch")
            nc.gpsimd.memset(scratch[:], 0.0)
            warm_ps = psum.tile([128, TF], F32, tag="warm", bufs=1)
            for _ in range(N_WARMUP_MM):
                nc.tensor.matmul(warm_ps[:, :], lhsT=scratch[:, :128],
                                 rhs=scratch[:, :], start=True, stop=True)

            xts = {}
            for img in range(IMGS):
                xa = xin.tile([C_IN, XA_R1 - XA_R0, WP], BF16, tag="xa")
                nc.sync.dma_start(out=xa[:], in_=xp[img, :, XA_R0:XA_R1])
                if img == 0:
                    # w directly after xa0: the first matmul's cumulative
                    # DMA-sem wait then covers exactly [xa0, w]
                    w_sb = consts.tile([C_IN, KH * KW, C_OUT], BF16, tag="w")
                    nc.sync.dma_start(out=w_sb[:], in_=w)
                xb = xin.tile([C_IN, XB_R1 - XB_R0, WP], BF16, tag="xb")
                nc.sync.dma_start(out=xb[:], in_=xp[img, :, XB_R0:XB_R1])
                xts[img] = (xa, xb)
                if img == 0:
                    b_sb = consts.tile([128, NH], F32, tag="b")
                    nc.sync.dma_start(out=b_sb[:], in_=b)

            for img in range(IMGS):
                xa, xb = xts[img]
                ots = [outp.tile([128, H * W], F32, tag=f"ot{h}",
                                 name=f"ot{img}_{h}")
                       for h in range(NH)]
                for t in range(NT):
                    if t < T_SPLIT:
                        src, r_off = xa, XA_R0
                    else:
                        src, r_off = xb, XB_R0
                    r0 = RPT * t - r_off
                    for h in range(NH):
                        pt = psum.tile([128, TF], F32, tag="pt")
                        for k in range(KH * KW):
                            kh, kw = divmod(k, KW)
                            nc.tensor.matmul(
                                pt[:, :],
                                lhsT=w_sb[:, k, h * 128:(h + 1) * 128],
                                rhs=src[:, r0 + kh:r0 + kh + RPT, kw:kw + W],
                                start=(k == 0),
                                stop=(k == KH * KW - 1),
                            )
                        nc.vector.tensor_scalar_add(
                            out=ots[h][:, t * TF:(t + 1) * TF],
                            in0=pt[:, :],
                            scalar1=b_sb[:, h:h + 1],
                        )
                        nc.sync.dma_start(
                            out=out_v[img, h * 128:(h + 1) * 128,
                                      t * TF:(t + 1) * TF],
                            in_=ots[h][:, t * TF:(t + 1) * TF])
    nc.compile()
    return nc


def get_program():
    if "nc" not in _CACHE:
        _CACHE["nc"] = _build_program()
    return _CACHE["nc"]


def make_in_maps(x, weight, bias):
    import ml_dtypes

    BF = ml_dtypes.bfloat16
    x = np.asarray(x)
    weight = np.asarray(weight)
    bias = np.asarray(bias, dtype=np.float32)

    xpad = np.zeros((N, C_IN, HP, WP), dtype=BF)
    xpad[:, :, 1:1 + H, 1:1 + W] = x.astype(BF)
    w_t = np.ascontiguousarray(
        weight.astype(np.float32).transpose(1, 2, 3, 0)
        .reshape(C_IN, KH * KW, C_OUT).astype(BF))
    b2 = np.ascontiguousarray(bias.reshape(NH, 128).T)

    return [
        {
            "xp": np.ascontiguousarray(xpad[i * IMGS:(i + 1) * IMGS]),
            "w": w_t,
            "b": b2,
        }
        for i in range(N_CORES)
    ]


def kernel(x, weight, bias):
    from concourse.bass_utils import run_bass_kernel_spmd

    nc = get_program()
    in_maps = make_in_maps(x, weight, bias)
    res = run_bass_kernel_spmd(nc, in_maps, core_ids=list(range(N_CORES)))
    return np.concatenate([res.results[i]["out"] for i in range(N_CORES)],
                          axis=0)
